# revision 57
# baseline (speedup 1.0000x reference)
"""BoundaryFluxAttention TRN2 kernel.

Distribution (8 cores): data-parallel over batch (B=2) x tensor-parallel over
heads (16 heads -> 4 groups of 4). Core c handles batch c//4, head group c%4.
Each core computes a partial output y_c = softmax-attention(its 4 heads) @ W_out
rows for those heads; the host sums the 4 partials per batch and adds b_out.

Per-core pipeline (T=2048, D=1024, 4 heads of hd=64), hand-pipelined emission:
  A:  QK^T projection qkt[db] [128, T] bf16 = (W slice)^T @ xT (bf16 matmuls);
      x arrives pre-transposed bf16 from the host. Scale hd^-0.5 folded in W_q.
      Q-projections for t-blocks 2/3 are deferred into later groups as PE
      fillers (phase 2 is ACT-paced, so the PE has slack there).
  B:  V projection in natural [T, 256] layout -> vsb [128, kb, h, 65] bf16
      with a ones column at index 64 (denominator accumulates in the same
      matmul as O'^T).
  C:  S^T tiles [128k, 1024(2 heads)] = K_h^T.T @ Q_h^T, heads row-tiled
      (K=64 at partition offsets 0/64), bf16 row-group concurrency.
  exp: ScalarE, per-partition bias = boundary*0.1, bf16 out. This is the
      pacing engine for phase 2 (~1.1us per k-block); everything else is
      scheduled around keeping it fed.
  D:  O'^T [65, 512] += V'_h.T @ P_h^T over k; row 64 = softmax denominator.
      Accumulators alternate pools per group (even groups: psAcc; odd
      groups: two psG slots) so a new group's accumulation never waits on
      the previous group's stage-out copies.
  norm: stage O' to SBUF per pair as soon as its group ends, then
      reciprocal_approx_fast on DVE (+cast to bf16) immediately - so the
      K=1 broadcast matmuls (emitted as early-kb fillers of the next even
      group) never stall the PE. Fused multiply writes bf16 ot.
      Odd heads shifted to partitions 64..127 via SBUF->SBUF DMA.
  E:  y = OT_pair @ W_out slice (bf16), emitted as late-kb fillers;
      y partials returned bf16, summed on host in fp32.
"""

import numpy as np
import ml_dtypes

import concourse.bass as bass  # noqa: F401
import concourse.mybir as mybir
import concourse.tile as tile
from concourse import bacc

F32 = mybir.dt.float32
F32R = mybir.dt.float32r
BF16 = mybir.dt.bfloat16
EXP = mybir.ActivationFunctionType.Exp

T = 2048
D = 1024
HPC = 4          # heads per core
HD = 64
NKB = T // 128   # 16 k/t blocks of 128
NQB = T // 512   # 4 q blocks of 512
NCH = D // 128   # 8 contraction chunks
SCALE = HD ** -0.5
BIAS_COEF = 0.1

_NC_CACHE = {}


def _build_nc(with_qkv_bias=True):
    nc = bacc.Bacc("TRN2", target_bir_lowering=False)

    xt_d = nc.declare_dram_parameter("xt", [D, T], BF16, isOutput=False)
    wqk_d = nc.declare_dram_parameter("wqk", [D, 512], BF16, isOutput=False)
    bqk_d = nc.declare_dram_parameter("bqk", [1, 512], BF16, isOutput=False)
    wv_d = nc.declare_dram_parameter("wv", [D, 256], BF16, isOutput=False)
    bv_d = nc.declare_dram_parameter("bv", [1, 256], BF16, isOutput=False)
    wo_d = nc.declare_dram_parameter("wo", [256, D], BF16, isOutput=False)
    bs_d = nc.declare_dram_parameter("bs", [128, NKB], F32, isOutput=False)
    ones_d = nc.declare_dram_parameter("ones", [1, 512], BF16, isOutput=False)
    ones65_d = nc.declare_dram_parameter("ones65", [65, 64], BF16, isOutput=False)
    y_d = nc.declare_dram_parameter("y", [T, D], BF16, isOutput=True)

    with tile.TileContext(nc) as tc:
        with (
            tc.tile_pool(name="const", bufs=1) as constp,
            tc.tile_pool(name="wts", bufs=1) as wts,
            tc.tile_pool(name="big", bufs=1) as bigp,
            tc.tile_pool(name="pt", bufs=4) as ptp,
            tc.tile_pool(name="ptpre", bufs=8) as ptpre,
            tc.tile_pool(name="norm", bufs=1) as normp,
            tc.tile_pool(name="stg", bufs=2) as stgp,
            tc.tile_pool(name="ysb", bufs=3) as ypool,
            tc.tile_pool(name="psG", bufs=2, space="PSUM") as psG,
            tc.tile_pool(name="psS", bufs=2, space="PSUM") as psS,
            tc.tile_pool(name="psAcc", bufs=1, space="PSUM") as psAcc,
        ):
            # Weights first (scalar DMA queue), one tile per contraction
            # chunk so the first LDWEIGHTS waits only on its own chunk.
            wqk_chunks_d = wqk_d.rearrange("(c p) n -> c p n", p=128)
            wqk_c = []
            for c in range(NCH):
                t_ = wts.tile([128, 512], BF16, tag=f"wqk{c}", name=f"wqk{c}")
                nc.scalar.dma_start(t_[:], wqk_chunks_d[c])
                wqk_c.append(t_)
            wv_chunks_d = wv_d.rearrange("(c p) n -> c p n", p=128)
            wv_c = []
            for c in range(NCH):
                t_ = wts.tile([128, 256], BF16, tag=f"wv{c}", name=f"wv{c}")
                nc.scalar.dma_start(t_[:], wv_chunks_d[c])
                wv_c.append(t_)
            bs_sb = constp.tile([128, NKB], F32, tag="bs")
            nc.scalar.dma_start(bs_sb[:], bs_d[:])
            ones = constp.tile([1, 512], BF16, tag="ones")
            bqk_sb = wts.tile([1, 512], BF16, tag="bqk")
            bv_sb = wts.tile([1, 256], BF16, tag="bv")
            if with_qkv_bias:
                nc.scalar.dma_start(ones[:], ones_d[:])
                nc.scalar.dma_start(bqk_sb[:], bqk_d[:])
                nc.scalar.dma_start(bv_sb[:], bv_d[:])
            # W_out and the broadcast ones aren't read until the first E /
            # norm (~half-way in), so their DMAs are issued mid-phase-1 to
            # keep the early HBM bandwidth for x^T and W_qk/W_v.
            wo_sb = wts.tile([128, 2, D], BF16, tag="wo")
            ones65 = constp.tile([65, 64], BF16, tag="ones65")

            # x^T per t-block (sync queue) so stage A(0) starts after 1MB.
            xt_chunks = xt_d.rearrange("(c p) t -> c p t", p=128)
            xT_t = []
            for tb in range(4):
                t_ = bigp.tile([128, NCH, 512], BF16, tag=f"xT{tb}", name=f"xT{tb}")
                for c in range(NCH):
                    nc.sync.dma_start(
                        t_[:, c, :],
                        xt_chunks[c][:, tb * 512:(tb + 1) * 512],
                    )
                xT_t.append(t_)

            qkt = [
                bigp.tile([128, T], BF16, tag=f"qkt{db}", name=f"qkt{db}")
                for db in range(4)
            ]
            vsb = bigp.tile([128, NKB, HPC, 65], BF16, tag="vsb", name="vsb_v10")
            nc.vector.memset(vsb[:, :, :, 64:65], 1.0)
            ot = [
                bigp.tile([128, T], BF16, tag=f"ot{pi}", name=f"ot{pi}")
                for pi in range(2)
            ]
            y_rows = y_d.rearrange("(n p) d -> n p d", p=128)

            # ---------------- emission helpers ----------------
            def emit_A_db(tb, db):
                ps = psG.tile([128, 512], F32, tag="gp", name=f"qk{tb}_{db}")
                for c in range(NCH):
                    nc.tensor.matmul(
                        ps[:],
                        wqk_c[c][:, db * 128:(db + 1) * 128],
                        xT_t[tb][:, c, :],
                        start=(c == 0),
                        stop=(not with_qkv_bias and c == NCH - 1),
                    )
                if with_qkv_bias:
                    nc.tensor.matmul(
                        ps[:],
                        bqk_sb[0:1, db * 128:(db + 1) * 128],
                        ones[0:1, :],
                        start=False,
                        stop=True,
                    )
                nc.vector.tensor_copy(qkt[db][:, tb * 512:(tb + 1) * 512], ps[:])

            def make_qproj_chain(tb, db, tag):
                # Deferred Q-projection chain for one (t-block, db), split
                # into single-matmul chunks. Runs inside an odd group, whose
                # psAcc slots sit idle (even groups own the accumulators
                # there), so it never competes with psG fillers.
                state = {}

                def ch(c):
                    if c == 0:
                        state["ps"] = psAcc.tile(
                            [128, 512], F32, tag=tag, name=f"qkd{tb}_{db}",
                        )
                    nc.tensor.matmul(
                        state["ps"][:],
                        wqk_c[c][:, db * 128:(db + 1) * 128],
                        xT_t[tb][:, c, :],
                        start=(c == 0),
                        stop=(not with_qkv_bias and c == NCH - 1),
                    )
                    if c == NCH - 1:
                        if with_qkv_bias:
                            nc.tensor.matmul(
                                state["ps"][:],
                                bqk_sb[0:1, db * 128:(db + 1) * 128],
                                ones[0:1, :],
                                start=False,
                                stop=True,
                            )
                        nc.vector.tensor_copy(
                            qkt[db][:, tb * 512:(tb + 1) * 512], state["ps"][:]
                        )

                return [lambda c=c: ch(c) for c in range(NCH)]

            def emit_B(tb):
                for j in range(4):
                    kb = tb * 4 + j
                    ps = psG.tile([128, 256], F32, tag="gp", name=f"v{kb}")
                    for c in range(NCH):
                        nc.tensor.matmul(
                            ps[:],
                            xT_t[tb][:, c, j * 128:(j + 1) * 128],
                            wv_c[c][:],
                            start=(c == 0),
                            stop=(not with_qkv_bias and c == NCH - 1),
                        )
                    if with_qkv_bias:
                        nc.tensor.matmul(
                            ps[:], ones[0:1, 0:128], bv_sb[:], start=False, stop=True
                        )
                    nc.vector.tensor_copy(
                        vsb[:, kb, :, 0:64],
                        ps[:].rearrange("p (h c) -> p h c", h=HPC),
                    )

            def emit_S_exp(qb, pi, kb, pool=None):
                qdb, kdb = pi, 2 + pi
                s01 = psS.tile([128, 1024], F32, tag="s01", name=f"s{qb}_{pi}_{kb}")
                nc.tensor.matmul(
                    s01[:, 0:512],
                    qkt[kdb][0:64, kb * 128:(kb + 1) * 128],
                    qkt[qdb][0:64, qb * 512:(qb + 1) * 512],
                )
                nc.tensor.matmul(
                    s01[:, 512:1024],
                    qkt[kdb][64:128, kb * 128:(kb + 1) * 128],
                    qkt[qdb][64:128, qb * 512:(qb + 1) * 512],
                )
                p01 = (pool or ptp).tile(
                    [128, 1024], BF16, tag="p01", name=f"p{qb}_{pi}_{kb}"
                )
                nc.scalar.activation(p01[:], s01[:], EXP, bias=bs_sb[:, kb:kb + 1])
                return p01

            def emit_D(qb, pi, kb, p01, osA, osB):
                nc.tensor.matmul(
                    osA[:], vsb[:, kb, 2 * pi, :], p01[:, 0:512],
                    start=(kb == 0), stop=(kb == NKB - 1),
                )
                nc.tensor.matmul(
                    osB[:], vsb[:, kb, 2 * pi + 1, :], p01[:, 512:1024],
                    start=(kb == 0), stop=(kb == NKB - 1),
                )

            stgq_tiles = {}
            rec_tiles = {}

            def emit_stage_out_rec(qb, pi, osA, osB, final=False):
                # Stage O' out of PSUM immediately so the accumulator slots
                # free fast, and compute this pair's 1/denominator right
                # away (DVE approx + bf16 cast) so the broadcast matmuls
                # later never wait on it.
                if pi == 0:
                    stgq_tiles[qb] = stgp.tile(
                        [65, 4, 512], F32, tag="stgq", name=f"stgq{qb}"
                    )
                    rec_tiles[qb] = (
                        normp.tile([1, 4, 512], BF16, tag="rec", name=f"rec{qb}"),
                        normp.tile([1, 4, 512], F32, tag="rec32",
                                   name=f"rec32_{qb}"),
                        normp.tile([1, 4, 512], F32, tag="den0",
                                   name=f"den0_{qb}"),
                    )
                stgq = stgq_tiles[qb]
                rec, rec32, den0 = rec_tiles[qb]
                if final:
                    # Tail epilogue: ACT is idle, split the copies across
                    # engines (Copy shares Exp's table set: no table load).
                    nc.scalar.copy(stgq[:, 2 * pi + 0, :], osA[:])
                else:
                    nc.vector.tensor_copy(stgq[:, 2 * pi + 0, :], osA[:])
                nc.vector.tensor_copy(stgq[:, 2 * pi + 1, :], osB[:])
                js = slice(2 * pi, 2 * pi + 2)
                # Shift the denominator row to partition 0 (the custom DVE
                # reciprocal is fed a plain contiguous partition-0 AP).
                nc.sync.dma_start(den0[0:1, js, :], stgq[64:65, js, :])
                nc.vector.reciprocal_approx_fast(
                    rec32[0:1, js, :], den0[0:1, js, :]
                )
                with nc.allow_low_precision(
                    reason="per-column softmax denominators; bf16 scale "
                    "error is ~0.4%, well inside the accuracy budget"
                ):
                    nc.vector.tensor_copy(rec[0:1, js, :], rec32[0:1, js, :])

            def make_bcmul_pair(qb, pi, ps_pool=None, ps_tag="gp"):
                # Partition-broadcast of 1/denom (K=1 matmul) + fused
                # normalize-multiply, one closure per head of the pair.
                stgq = stgq_tiles[qb]
                rec = rec_tiles[qb][0]
                if pi == 1:
                    stgq_tiles.pop(qb)
                    rec_tiles.pop(qb)
                cols = slice(qb * 512, (qb + 1) * 512)

                def mk(j):
                    parity = j % 2

                    def ch():
                        bc_ps = (ps_pool or psG).tile(
                            [64, 512], F32, tag=ps_tag, name=f"bc{qb}_{j}"
                        )
                        nc.tensor.matmul(
                            bc_ps[:], ones65[0:1, 0:64], rec[0:1, j, :]
                        )
                        if parity == 0:
                            nc.vector.tensor_mul(
                                ot[pi][0:64, cols], stgq[0:64, j, :], bc_ps[0:64, :]
                            )
                        else:
                            stag = normp.tile([64, 512], BF16, tag="stag")
                            nc.vector.tensor_mul(
                                stag[:], stgq[0:64, j, :], bc_ps[0:64, :]
                            )
                            nc.sync.dma_start(ot[pi][64:128, cols], stag[:])
                    return ch

                return [mk(2 * pi), mk(2 * pi + 1)]

            def make_E_chunks(qb, tail=False):
                # Stage E for one q-block, sliced into 16 small closures.
                # In the tail (no attention left) alternate t-blocks between
                # the psG pair and the freed psAcc pair, and split the PSUM
                # drain copies across ScalarE/DVE, so two chains overlap.
                chunks = []
                for j in range(4):
                    tb = qb * 4 + j
                    state = {}
                    on_acc = tail and j % 2 == 1

                    def c0(tb=tb, state=state, on_acc=on_acc):
                        state["ysb"] = ypool.tile(
                            [128, D], BF16, tag="ysb", name=f"ysb{tb}"
                        )
                        if on_acc:
                            state["yps"] = [
                                psAcc.tile([128, 512], F32, tag=t_,
                                           name=f"yps{tb}_{t_}")
                                for t_ in ("osA", "osB")
                            ]
                        else:
                            state["yps"] = [
                                psG.tile([128, 512], F32, tag="gp",
                                         name=f"yps{tb}_{nb}")
                                for nb in range(2)
                            ]
                        nc.tensor.matmul(
                            state["yps"][0][:],
                            ot[0][:, tb * 128:(tb + 1) * 128],
                            wo_sb[:, 0, 0:512],
                            start=True, stop=False,
                        )

                    def c1(tb=tb, state=state):
                        nc.tensor.matmul(
                            state["yps"][1][:],
                            ot[0][:, tb * 128:(tb + 1) * 128],
                            wo_sb[:, 0, 512:1024],
                            start=True, stop=False,
                        )

                    def c2(tb=tb, state=state):
                        nc.tensor.matmul(
                            state["yps"][0][:],
                            ot[1][:, tb * 128:(tb + 1) * 128],
                            wo_sb[:, 1, 0:512],
                            start=False, stop=True,
                        )
                        if tail:
                            nc.scalar.copy(
                                state["ysb"][:, 0:512], state["yps"][0][:]
                            )
                        else:
                            nc.vector.tensor_copy(
                                state["ysb"][:, 0:512], state["yps"][0][:]
                            )

                    def c3(tb=tb, state=state):
                        nc.tensor.matmul(
                            state["yps"][1][:],
                            ot[1][:, tb * 128:(tb + 1) * 128],
                            wo_sb[:, 1, 512:1024],
                            start=False, stop=True,
                        )
                        nc.vector.tensor_copy(
                            state["ysb"][:, 512:1024], state["yps"][1][:]
                        )
                        nc.sync.dma_start(y_rows[tb], state["ysb"][:])

                    chunks += [c0, c1, c2, c3]
                return chunks

            # ---------------- pipelined emission ----------------
            # Phase 1 (= group 0, accums in psAcc): K-projections for all
            # t-blocks, V for all, Q-projections for t-blocks 0/1, with the
            # (q0, pair0) k-sweep interleaved so ScalarE ramps early.
            groups = [(0, 1)] + [(qb, pi) for qb in range(1, NQB) for pi in range(2)]

            osA = psAcc.tile([65, 512], F32, tag="osA", name="osA0_0")
            osB = psAcc.tile([65, 512], F32, tag="osB", name="osB0_0")
            pre01 = {}
            for tb in range(4):
                if tb == 2:
                    nc.scalar.dma_start(
                        wo_sb[:], wo_d.rearrange("(c p) n -> p c n", p=128)
                    )
                    nc.scalar.dma_start(ones65[:], ones65_d[:])
                emit_A_db(tb, 2)
                emit_A_db(tb, 3)
                emit_B(tb)
                if tb == 0:
                    emit_A_db(tb, 0)
                    emit_A_db(tb, 1)
                for kb in range(4 * tb, 4 * tb + 4):
                    p01 = emit_S_exp(0, 0, kb)
                    emit_D(0, 0, kb, p01, osA, osB)
                # Pre-compute two of group (0,1)'s S+exp per t-block into
                # dedicated single-use buffers: ScalarE is otherwise starved
                # here, and group (0,1) then starts with half its exps done.
                for j in (2 * tb, 2 * tb + 1):
                    pre01[j] = emit_S_exp(0, 1, j, pool=ptpre)
            preS = [pre01[0], pre01[1]]
            pending = (0, 0, osA, osB)

            # Phase 2: groups G=1..7 alternate accumulator pools (odd G:
            # two psG slots; even G: psAcc; the last group also psAcc so
            # its pair-0 normalize can run as fillers inside it). Even
            # groups carry fillers: broadcast+multiply for the previous
            # q-block, deferred Q-projections, then that q-block's E.
            for gi, (qb, pi) in enumerate(groups):
                G = gi + 1
                last = gi == len(groups) - 1
                p01s = {0: preS[0], 1: preS[1]}
                if gi == 0:
                    p01s.update(pre01)
                pqb, ppi, posA, posB = pending
                emit_stage_out_rec(pqb, ppi, posA, posB)
                fillers = []
                if ppi == 1:
                    fillers += [(2, ch) for ch in make_bcmul_pair(pqb, 0)]
                    fillers += [(4, ch) for ch in make_bcmul_pair(pqb, 1)]
                    fillers += [(7, ch) for ch in make_E_chunks(pqb)]
                elif G == 1:
                    fillers += [(2, ch) for ch in make_qproj_chain(1, 0, "osA")]
                    fillers += [(2, ch) for ch in make_qproj_chain(1, 1, "osB")]
                elif G == 3:
                    fillers += [(2, ch) for ch in make_qproj_chain(2, 0, "osA")]
                    fillers += [(2, ch) for ch in make_qproj_chain(2, 1, "osB")]
                elif G == 5:
                    fillers += [(2, ch) for ch in make_qproj_chain(3, 0, "osA")]
                    fillers += [(2, ch) for ch in make_qproj_chain(3, 1, "osB")]
                if last:
                    fillers += [(3, ch) for ch in make_bcmul_pair(qb, 0)]
                    tail_chunks = make_E_chunks(qb, tail=True)
                    fillers += [(12, tail_chunks[0]), (13, tail_chunks[1])]
                if G % 2 == 0 or last:
                    osA = psAcc.tile([65, 512], F32, tag="osA", name=f"osA{qb}_{pi}")
                    osB = psAcc.tile([65, 512], F32, tag="osB", name=f"osB{qb}_{pi}")
                else:
                    osA = psG.tile([65, 512], F32, tag="gp", name=f"osA{qb}_{pi}")
                    osB = psG.tile([65, 512], F32, tag="gp", name=f"osB{qb}_{pi}")
                # Software-pipelined two deep: each iteration emits D(kb)
                # then S(kb+2), and only then fillers — so filler matmuls
                # absorb the D-waits and never sit in front of the next S
                # in the in-order PE queue (they would gap the exp stream).
                emit_D(qb, pi, 0, p01s.pop(0), osA, osB)
                if 2 not in p01s:
                    p01s[2] = emit_S_exp(qb, pi, 2)
                for kb in range(1, NKB):
                    emit_D(qb, pi, kb, p01s.pop(kb), osA, osB)
                    nk = kb + 2
                    if nk < NKB:
                        if nk not in p01s:
                            p01s[nk] = emit_S_exp(qb, pi, nk)
                    elif not last:
                        preS[nk - NKB] = emit_S_exp(*groups[gi + 1], nk - NKB)
                    if kb == NKB - 1:
                        break
                    budget = 2 if len(fillers) > (NKB - 2 - kb) else 1
                    while budget and fillers and fillers[0][0] <= kb:
                        fillers.pop(0)[1]()
                        budget -= 1
                pending = (qb, pi, osA, osB)
                for _, ch in fillers:
                    ch()

            qb, pi, osA, osB = pending
            emit_stage_out_rec(qb, pi, osA, osB, final=True)
            # First-half chunks only need ot[0] (pair-0 normalize ran inside
            # the last group; tb0's first halves already went out as its
            # fillers), so they fill the PE while DVE derives the pair-1
            # reciprocals; then the pair-1 broadcast+mul, then the second
            # halves, two chains overlapped across the PSUM pools.
            for k in (4, 5):
                tail_chunks[k]()
            for ch in make_bcmul_pair(qb, 1, ps_pool=psS, ps_tag="s01"):
                ch()
            for k in (2, 3, 8, 9, 6, 7, 12, 13, 10, 11, 14, 15):
                tail_chunks[k]()

    nc.compile()
    return nc


def _get_nc(with_qkv_bias=True):
    key = ("nc", with_qkv_bias)
    if key not in _NC_CACHE:
        _NC_CACHE[key] = _build_nc(with_qkv_bias)
    return _NC_CACHE[key]


def _make_in_maps(x, boundary_score, W_qkv, b_qkv, W_out):
    bf16 = ml_dtypes.bfloat16
    x = np.asarray(x, np.float32)
    boundary_score = np.asarray(boundary_score, np.float32)
    W_qkv = np.asarray(W_qkv, np.float32)
    b_qkv = np.asarray(b_qkv, np.float32)
    W_out = np.asarray(W_out, np.float32)

    Wq, Wk, Wv = W_qkv[:, :D], W_qkv[:, D:2 * D], W_qkv[:, 2 * D:]
    bq, bk, bv = b_qkv[:D], b_qkv[D:2 * D], b_qkv[2 * D:]
    ones = np.ones((1, 512), bf16)
    ones65 = np.ones((65, 64), bf16)
    xts = [np.ascontiguousarray(x[b].T.astype(bf16)) for b in range(x.shape[0])]

    in_maps = []
    for c in range(8):
        b, g = divmod(c, 4)
        lo, hi = 256 * g, 256 * (g + 1)
        wqk = np.ascontiguousarray(
            np.concatenate([Wq[:, lo:hi] * SCALE, Wk[:, lo:hi]], axis=1).astype(bf16)
        )
        bqk = np.concatenate([bq[lo:hi] * SCALE, bk[lo:hi]])[None].astype(bf16)
        wv = np.ascontiguousarray(Wv[:, lo:hi].astype(bf16))
        bvv = np.ascontiguousarray(bv[lo:hi][None].astype(bf16))
        wo = np.ascontiguousarray(W_out[lo:hi, :].astype(bf16))
        bs = np.ascontiguousarray(
            (boundary_score[b] * BIAS_COEF).reshape(NKB, 128).T
        )
        in_maps.append(
            dict(
                xt=xts[b], wqk=wqk, bqk=np.ascontiguousarray(bqk),
                wv=wv, bv=bvv, wo=wo, bs=bs, ones=ones, ones65=ones65,
            )
        )
    return in_maps


def kernel(x, boundary_score, W_qkv, b_qkv, W_out, b_out):
    from concourse.bass_utils import run_bass_kernel_spmd

    x = np.asarray(x, np.float32)
    B = x.shape[0]
    in_maps = _make_in_maps(x, boundary_score, W_qkv, b_qkv, W_out)
    nc = _get_nc(with_qkv_bias=bool(np.any(np.asarray(b_qkv))))
    res = run_bass_kernel_spmd(nc, in_maps, list(range(8))).results
    out = np.zeros((B, T, D), np.float32)
    for c in range(8):
        out[c // 4] += np.asarray(res[c]["y"], np.float32)
    out += np.asarray(b_out, np.float32)
    return out


# revision 58
# speedup vs baseline: 1.1805x; 1.1805x over previous
"""BoundaryFluxAttention TRN2 kernel.

Distribution (8 cores): data-parallel over batch (B=2) x tensor-parallel over
heads (16 heads -> 4 groups of 4). Core c handles batch c//4, head group c%4.
Each core computes a partial output y_c = softmax-attention(its 4 heads) @ W_out
rows for those heads; the host sums the 4 partials per batch and adds b_out.

Per-core pipeline (T=2048, D=1024, 4 heads of hd=64), hand-pipelined emission:
  A:  QK^T projection qkt[db] [128, T] bf16 = (W slice)^T @ xT (bf16 matmuls);
      x arrives pre-transposed bf16 from the host. Scale hd^-0.5 folded in W_q.
      Q-projections for t-blocks 2/3 are deferred into later groups as PE
      fillers (phase 2 is ACT-paced, so the PE has slack there).
  B:  V projection in natural [T, 256] layout -> vsb [128, kb, h, 65] bf16
      with a ones column at index 64 (denominator accumulates in the same
      matmul as O'^T).
  C:  S^T tiles [128k, 1024(2 heads)] = K_h^T.T @ Q_h^T, heads row-tiled
      (K=64 at partition offsets 0/64), bf16 row-group concurrency.
  exp: ScalarE, per-partition bias = boundary*0.1, bf16 out. This is the
      pacing engine for phase 2 (~1.1us per k-block); everything else is
      scheduled around keeping it fed.
  D:  O'^T [65, 512] += V'_h.T @ P_h^T over k; row 64 = softmax denominator.
      Accumulators alternate pools per group (even groups: psAcc; odd
      groups: two psG slots) so a new group's accumulation never waits on
      the previous group's stage-out copies.
  norm: stage O' to SBUF per pair as soon as its group ends, then
      reciprocal_approx_fast on DVE (+cast to bf16) immediately - so the
      K=1 broadcast matmuls (emitted as early-kb fillers of the next even
      group) never stall the PE. Fused multiply writes bf16 ot.
      Odd heads shifted to partitions 64..127 via SBUF->SBUF DMA.
  E:  y = OT_pair @ W_out slice (bf16), emitted as late-kb fillers;
      y partials returned bf16, summed on host in fp32.
"""

import numpy as np
import ml_dtypes

import concourse.bass as bass  # noqa: F401
import concourse.mybir as mybir
import concourse.tile as tile
from concourse import bacc

F32 = mybir.dt.float32
F32R = mybir.dt.float32r
BF16 = mybir.dt.bfloat16
EXP = mybir.ActivationFunctionType.Exp

T = 2048
D = 1024
HPC = 4          # heads per core
HD = 64
NKB = T // 128   # 16 k/t blocks of 128
NQB = T // 512   # 4 q blocks of 512
NCH = D // 128   # 8 contraction chunks
SCALE = HD ** -0.5
BIAS_COEF = 0.1

_NC_CACHE = {}


def _build_nc(with_qkv_bias=True):
    nc = bacc.Bacc("TRN2", target_bir_lowering=False)

    xt_d = nc.declare_dram_parameter("xt", [D, T], BF16, isOutput=False)
    wqk_d = nc.declare_dram_parameter("wqk", [D, 512], BF16, isOutput=False)
    bqk_d = nc.declare_dram_parameter("bqk", [1, 512], BF16, isOutput=False)
    wv_d = nc.declare_dram_parameter("wv", [D, 256], BF16, isOutput=False)
    bv_d = nc.declare_dram_parameter("bv", [1, 256], BF16, isOutput=False)
    wo_d = nc.declare_dram_parameter("wo", [256, D], BF16, isOutput=False)
    bs_d = nc.declare_dram_parameter("bs", [128, NKB], F32, isOutput=False)
    ones_d = nc.declare_dram_parameter("ones", [1, 512], BF16, isOutput=False)
    ones65_d = nc.declare_dram_parameter("ones65", [65, 64], BF16, isOutput=False)
    y_d = nc.declare_dram_parameter("y", [T, D], BF16, isOutput=True)

    with tile.TileContext(nc) as tc:
        with (
            tc.tile_pool(name="const", bufs=1) as constp,
            tc.tile_pool(name="wts", bufs=1) as wts,
            tc.tile_pool(name="big", bufs=1) as bigp,
            tc.tile_pool(name="pt", bufs=4) as ptp,
            tc.tile_pool(name="norm", bufs=1) as normp,
            tc.tile_pool(name="stg", bufs=2) as stgp,
            tc.tile_pool(name="ysb", bufs=3) as ypool,
            tc.tile_pool(name="psG", bufs=2, space="PSUM") as psG,
            tc.tile_pool(name="psS", bufs=2, space="PSUM") as psS,
            tc.tile_pool(name="psAcc", bufs=1, space="PSUM") as psAcc,
        ):
            # Weights first (scalar DMA queue), one tile per contraction
            # chunk so the first LDWEIGHTS waits only on its own chunk.
            wqk_chunks_d = wqk_d.rearrange("(c p) n -> c p n", p=128)
            wqk_c = []
            for c in range(NCH):
                t_ = wts.tile([128, 512], BF16, tag=f"wqk{c}", name=f"wqk{c}")
                nc.scalar.dma_start(t_[:], wqk_chunks_d[c])
                wqk_c.append(t_)
            wv_chunks_d = wv_d.rearrange("(c p) n -> c p n", p=128)
            wv_c = []
            for c in range(NCH):
                t_ = wts.tile([128, 256], BF16, tag=f"wv{c}", name=f"wv{c}")
                nc.scalar.dma_start(t_[:], wv_chunks_d[c])
                wv_c.append(t_)
            bs_sb = constp.tile([128, NKB], F32, tag="bs")
            nc.scalar.dma_start(bs_sb[:], bs_d[:])
            ones = constp.tile([1, 512], BF16, tag="ones")
            bqk_sb = wts.tile([1, 512], BF16, tag="bqk")
            bv_sb = wts.tile([1, 256], BF16, tag="bv")
            if with_qkv_bias:
                nc.scalar.dma_start(ones[:], ones_d[:])
                nc.scalar.dma_start(bqk_sb[:], bqk_d[:])
                nc.scalar.dma_start(bv_sb[:], bv_d[:])
            # W_out and the broadcast ones aren't read until the first E /
            # norm (~half-way in), so their DMAs are issued mid-phase-1 to
            # keep the early HBM bandwidth for x^T and W_qk/W_v.
            wo_sb = wts.tile([128, 2, D], BF16, tag="wo")
            ones65 = constp.tile([65, 64], BF16, tag="ones65")

            # x^T per t-block (sync queue) so stage A(0) starts after 1MB.
            xt_chunks = xt_d.rearrange("(c p) t -> c p t", p=128)
            xT_t = []
            for tb in range(4):
                t_ = bigp.tile([128, NCH, 512], BF16, tag=f"xT{tb}", name=f"xT{tb}")
                for c in range(NCH):
                    nc.sync.dma_start(
                        t_[:, c, :],
                        xt_chunks[c][:, tb * 512:(tb + 1) * 512],
                    )
                xT_t.append(t_)

            qkt = [
                bigp.tile([128, T], BF16, tag=f"qkt{db}", name=f"qkt{db}")
                for db in range(4)
            ]
            vsb = bigp.tile([128, NKB, HPC, 65], BF16, tag="vsb", name="vsb_v10")
            nc.vector.memset(vsb[:, :, :, 64:65], 1.0)
            ot = [
                bigp.tile([128, T], BF16, tag=f"ot{pi}", name=f"ot{pi}")
                for pi in range(2)
            ]
            y_rows = y_d.rearrange("(n p) d -> n p d", p=128)

            # ---------------- emission helpers ----------------
            def emit_A_db(tb, db):
                ps = psG.tile([128, 512], F32, tag="gp", name=f"qk{tb}_{db}")
                for c in range(NCH):
                    nc.tensor.matmul(
                        ps[:],
                        wqk_c[c][:, db * 128:(db + 1) * 128],
                        xT_t[tb][:, c, :],
                        start=(c == 0),
                        stop=(not with_qkv_bias and c == NCH - 1),
                    )
                if with_qkv_bias:
                    nc.tensor.matmul(
                        ps[:],
                        bqk_sb[0:1, db * 128:(db + 1) * 128],
                        ones[0:1, :],
                        start=False,
                        stop=True,
                    )
                nc.vector.tensor_copy(qkt[db][:, tb * 512:(tb + 1) * 512], ps[:])

            def make_qproj_chain(tb, db, tag):
                # Deferred Q-projection chain for one (t-block, db), split
                # into single-matmul chunks. Runs inside an odd group, whose
                # psAcc slots sit idle (even groups own the accumulators
                # there), so it never competes with psG fillers.
                state = {}

                def ch(c):
                    if c == 0:
                        state["ps"] = psAcc.tile(
                            [128, 512], F32, tag=tag, name=f"qkd{tb}_{db}",
                        )
                    nc.tensor.matmul(
                        state["ps"][:],
                        wqk_c[c][:, db * 128:(db + 1) * 128],
                        xT_t[tb][:, c, :],
                        start=(c == 0),
                        stop=(not with_qkv_bias and c == NCH - 1),
                    )
                    if c == NCH - 1:
                        if with_qkv_bias:
                            nc.tensor.matmul(
                                state["ps"][:],
                                bqk_sb[0:1, db * 128:(db + 1) * 128],
                                ones[0:1, :],
                                start=False,
                                stop=True,
                            )
                        nc.vector.tensor_copy(
                            qkt[db][:, tb * 512:(tb + 1) * 512], state["ps"][:]
                        )

                return [lambda c=c: ch(c) for c in range(NCH)]

            def emit_B(tb):
                for j in range(4):
                    kb = tb * 4 + j
                    ps = psG.tile([128, 256], F32, tag="gp", name=f"v{kb}")
                    for c in range(NCH):
                        nc.tensor.matmul(
                            ps[:],
                            xT_t[tb][:, c, j * 128:(j + 1) * 128],
                            wv_c[c][:],
                            start=(c == 0),
                            stop=(not with_qkv_bias and c == NCH - 1),
                        )
                    if with_qkv_bias:
                        nc.tensor.matmul(
                            ps[:], ones[0:1, 0:128], bv_sb[:], start=False, stop=True
                        )
                    nc.vector.tensor_copy(
                        vsb[:, kb, :, 0:64],
                        ps[:].rearrange("p (h c) -> p h c", h=HPC),
                    )

            def emit_S_exp(qb, pi, kb):
                qdb, kdb = pi, 2 + pi
                s01 = psS.tile([128, 1024], F32, tag="s01", name=f"s{qb}_{pi}_{kb}")
                nc.tensor.matmul(
                    s01[:, 0:512],
                    qkt[kdb][0:64, kb * 128:(kb + 1) * 128],
                    qkt[qdb][0:64, qb * 512:(qb + 1) * 512],
                )
                nc.tensor.matmul(
                    s01[:, 512:1024],
                    qkt[kdb][64:128, kb * 128:(kb + 1) * 128],
                    qkt[qdb][64:128, qb * 512:(qb + 1) * 512],
                )
                p01 = ptp.tile([128, 1024], BF16, tag="p01", name=f"p{qb}_{pi}_{kb}")
                nc.scalar.activation(p01[:], s01[:], EXP, bias=bs_sb[:, kb:kb + 1])
                return p01

            def emit_D(qb, pi, kb, p01, osA, osB):
                nc.tensor.matmul(
                    osA[:], vsb[:, kb, 2 * pi, :], p01[:, 0:512],
                    start=(kb == 0), stop=(kb == NKB - 1),
                )
                nc.tensor.matmul(
                    osB[:], vsb[:, kb, 2 * pi + 1, :], p01[:, 512:1024],
                    start=(kb == 0), stop=(kb == NKB - 1),
                )

            stgq_tiles = {}
            rec_tiles = {}

            def emit_stage_out_rec(qb, pi, osA, osB, final=False):
                # Stage O' out of PSUM immediately so the accumulator slots
                # free fast, and compute this pair's 1/denominator right
                # away (DVE approx + bf16 cast) so the broadcast matmuls
                # later never wait on it.
                if pi == 0:
                    stgq_tiles[qb] = stgp.tile(
                        [65, 4, 512], F32, tag="stgq", name=f"stgq{qb}"
                    )
                    rec_tiles[qb] = (
                        normp.tile([1, 4, 512], BF16, tag="rec", name=f"rec{qb}"),
                        normp.tile([1, 4, 512], F32, tag="rec32",
                                   name=f"rec32_{qb}"),
                        normp.tile([1, 4, 512], F32, tag="den0",
                                   name=f"den0_{qb}"),
                    )
                stgq = stgq_tiles[qb]
                rec, rec32, den0 = rec_tiles[qb]
                if final:
                    # Tail epilogue: ACT is idle, split the copies across
                    # engines (Copy shares Exp's table set: no table load).
                    nc.scalar.copy(stgq[:, 2 * pi + 0, :], osA[:])
                else:
                    nc.vector.tensor_copy(stgq[:, 2 * pi + 0, :], osA[:])
                nc.vector.tensor_copy(stgq[:, 2 * pi + 1, :], osB[:])
                js = slice(2 * pi, 2 * pi + 2)
                # Shift the denominator row to partition 0 (the custom DVE
                # reciprocal is fed a plain contiguous partition-0 AP).
                nc.sync.dma_start(den0[0:1, js, :], stgq[64:65, js, :])
                nc.vector.reciprocal_approx_fast(
                    rec32[0:1, js, :], den0[0:1, js, :]
                )
                with nc.allow_low_precision(
                    reason="per-column softmax denominators; bf16 scale "
                    "error is ~0.4%, well inside the accuracy budget"
                ):
                    nc.vector.tensor_copy(rec[0:1, js, :], rec32[0:1, js, :])

            def make_bcmul_pair(qb, pi, ps_pool=None, ps_tag="gp"):
                # Partition-broadcast of 1/denom (K=1 matmul) + fused
                # normalize-multiply, one closure per head of the pair.
                stgq = stgq_tiles[qb]
                rec = rec_tiles[qb][0]
                if pi == 1:
                    stgq_tiles.pop(qb)
                    rec_tiles.pop(qb)
                cols = slice(qb * 512, (qb + 1) * 512)

                def mk(j):
                    parity = j % 2

                    def ch():
                        bc_ps = (ps_pool or psG).tile(
                            [64, 512], F32, tag=ps_tag, name=f"bc{qb}_{j}"
                        )
                        nc.tensor.matmul(
                            bc_ps[:], ones65[0:1, 0:64], rec[0:1, j, :]
                        )
                        if parity == 0:
                            nc.vector.tensor_mul(
                                ot[pi][0:64, cols], stgq[0:64, j, :], bc_ps[0:64, :]
                            )
                        else:
                            stag = normp.tile([64, 512], BF16, tag="stag")
                            nc.vector.tensor_mul(
                                stag[:], stgq[0:64, j, :], bc_ps[0:64, :]
                            )
                            nc.sync.dma_start(ot[pi][64:128, cols], stag[:])
                    return ch

                return [mk(2 * pi), mk(2 * pi + 1)]

            def make_E_chunks(qb, tail=False):
                # Stage E for one q-block, sliced into 16 small closures.
                # In the tail (no attention left) alternate t-blocks between
                # the psG pair and the freed psAcc pair, and split the PSUM
                # drain copies across ScalarE/DVE, so two chains overlap.
                chunks = []
                for j in range(4):
                    tb = qb * 4 + j
                    state = {}
                    on_acc = tail and j % 2 == 1

                    def c0(tb=tb, state=state, on_acc=on_acc):
                        state["ysb"] = ypool.tile(
                            [128, D], BF16, tag="ysb", name=f"ysb{tb}"
                        )
                        if on_acc:
                            state["yps"] = [
                                psAcc.tile([128, 512], F32, tag=t_,
                                           name=f"yps{tb}_{t_}")
                                for t_ in ("osA", "osB")
                            ]
                        else:
                            state["yps"] = [
                                psG.tile([128, 512], F32, tag="gp",
                                         name=f"yps{tb}_{nb}")
                                for nb in range(2)
                            ]
                        nc.tensor.matmul(
                            state["yps"][0][:],
                            ot[0][:, tb * 128:(tb + 1) * 128],
                            wo_sb[:, 0, 0:512],
                            start=True, stop=False,
                        )

                    def c1(tb=tb, state=state):
                        nc.tensor.matmul(
                            state["yps"][1][:],
                            ot[0][:, tb * 128:(tb + 1) * 128],
                            wo_sb[:, 0, 512:1024],
                            start=True, stop=False,
                        )

                    def c2(tb=tb, state=state):
                        nc.tensor.matmul(
                            state["yps"][0][:],
                            ot[1][:, tb * 128:(tb + 1) * 128],
                            wo_sb[:, 1, 0:512],
                            start=False, stop=True,
                        )
                        if tail:
                            nc.scalar.copy(
                                state["ysb"][:, 0:512], state["yps"][0][:]
                            )
                        else:
                            nc.vector.tensor_copy(
                                state["ysb"][:, 0:512], state["yps"][0][:]
                            )

                    def c3(tb=tb, state=state):
                        nc.tensor.matmul(
                            state["yps"][1][:],
                            ot[1][:, tb * 128:(tb + 1) * 128],
                            wo_sb[:, 1, 512:1024],
                            start=False, stop=True,
                        )
                        nc.vector.tensor_copy(
                            state["ysb"][:, 512:1024], state["yps"][1][:]
                        )
                        nc.sync.dma_start(y_rows[tb], state["ysb"][:])

                    chunks += [c0, c1, c2, c3]
                return chunks

            # ---------------- pipelined emission ----------------
            # Phase 1 (= group 0, accums in psAcc): K-projections for all
            # t-blocks, V for all, Q-projections for t-blocks 0/1, with the
            # (q0, pair0) k-sweep interleaved so ScalarE ramps early.
            groups = [(0, 1)] + [(qb, pi) for qb in range(1, NQB) for pi in range(2)]

            osA = psAcc.tile([65, 512], F32, tag="osA", name="osA0_0")
            osB = psAcc.tile([65, 512], F32, tag="osB", name="osB0_0")
            preS = [None, None]
            for tb in range(4):
                if tb == 2:
                    nc.scalar.dma_start(
                        wo_sb[:], wo_d.rearrange("(c p) n -> p c n", p=128)
                    )
                    nc.scalar.dma_start(ones65[:], ones65_d[:])
                emit_A_db(tb, 2)
                emit_A_db(tb, 3)
                emit_B(tb)
                if tb == 0:
                    emit_A_db(tb, 0)
                    emit_A_db(tb, 1)
                for kb in range(4 * tb, 4 * tb + 4):
                    p01 = emit_S_exp(0, 0, kb)
                    if kb == NKB - 1:
                        # Next group's first S goes out before the last D so
                        # ScalarE rolls across the boundary without a gap.
                        preS[0] = emit_S_exp(*groups[0], 0)
                    emit_D(0, 0, kb, p01, osA, osB)
            preS[1] = emit_S_exp(*groups[0], 1)
            pending = (0, 0, osA, osB)

            # Phase 2: groups G=1..7 alternate accumulator pools (odd G:
            # two psG slots; even G: psAcc; the last group also psAcc so
            # its pair-0 normalize can run as fillers inside it). Even
            # groups carry fillers: broadcast+multiply for the previous
            # q-block, deferred Q-projections, then that q-block's E.
            for gi, (qb, pi) in enumerate(groups):
                G = gi + 1
                last = gi == len(groups) - 1
                p01s = {0: preS[0], 1: preS[1]}
                pqb, ppi, posA, posB = pending
                emit_stage_out_rec(pqb, ppi, posA, posB)
                fillers = []
                if ppi == 1:
                    fillers += [(2, ch) for ch in make_bcmul_pair(pqb, 0)]
                    fillers += [(4, ch) for ch in make_bcmul_pair(pqb, 1)]
                    fillers += [(7, ch) for ch in make_E_chunks(pqb)]
                elif G == 1:
                    fillers += [(2, ch) for ch in make_qproj_chain(1, 0, "osA")]
                    fillers += [(2, ch) for ch in make_qproj_chain(1, 1, "osB")]
                elif G == 3:
                    fillers += [(2, ch) for ch in make_qproj_chain(2, 0, "osA")]
                    fillers += [(2, ch) for ch in make_qproj_chain(2, 1, "osB")]
                elif G == 5:
                    fillers += [(2, ch) for ch in make_qproj_chain(3, 0, "osA")]
                    fillers += [(2, ch) for ch in make_qproj_chain(3, 1, "osB")]
                if last:
                    fillers += [(3, ch) for ch in make_bcmul_pair(qb, 0)]
                    tail_chunks = make_E_chunks(qb, tail=True)
                    fillers += [(12, tail_chunks[0]), (13, tail_chunks[1])]
                if G % 2 == 0 or last:
                    osA = psAcc.tile([65, 512], F32, tag="osA", name=f"osA{qb}_{pi}")
                    osB = psAcc.tile([65, 512], F32, tag="osB", name=f"osB{qb}_{pi}")
                else:
                    osA = psG.tile([65, 512], F32, tag="gp", name=f"osA{qb}_{pi}")
                    osB = psG.tile([65, 512], F32, tag="gp", name=f"osB{qb}_{pi}")
                # Software-pipelined two deep: each iteration emits D(kb)
                # then S(kb+2), and only then fillers — so filler matmuls
                # absorb the D-waits and never sit in front of the next S
                # in the in-order PE queue (they would gap the exp stream).
                emit_D(qb, pi, 0, p01s.pop(0), osA, osB)
                p01s[2] = emit_S_exp(qb, pi, 2)
                for kb in range(1, NKB):
                    emit_D(qb, pi, kb, p01s.pop(kb), osA, osB)
                    nk = kb + 2
                    if nk < NKB:
                        p01s[nk] = emit_S_exp(qb, pi, nk)
                    elif not last:
                        preS[nk - NKB] = emit_S_exp(*groups[gi + 1], nk - NKB)
                    if kb == NKB - 1:
                        break
                    budget = 2 if len(fillers) > (NKB - 2 - kb) else 1
                    while budget and fillers and fillers[0][0] <= kb:
                        fillers.pop(0)[1]()
                        budget -= 1
                pending = (qb, pi, osA, osB)
                for _, ch in fillers:
                    ch()

            qb, pi, osA, osB = pending
            emit_stage_out_rec(qb, pi, osA, osB, final=True)
            # First-half chunks only need ot[0] (pair-0 normalize ran inside
            # the last group; tb0's first halves already went out as its
            # fillers), so they fill the PE while DVE derives the pair-1
            # reciprocals; then the pair-1 broadcast+mul, then the second
            # halves, two chains overlapped across the PSUM pools.
            for k in (4, 5):
                tail_chunks[k]()
            for ch in make_bcmul_pair(qb, 1, ps_pool=psS, ps_tag="s01"):
                ch()
            for k in (2, 3, 8, 9, 6, 7, 12, 13, 10, 11, 14, 15):
                tail_chunks[k]()

    nc.compile()
    return nc


def _get_nc(with_qkv_bias=True):
    key = ("nc", with_qkv_bias)
    if key not in _NC_CACHE:
        _NC_CACHE[key] = _build_nc(with_qkv_bias)
    return _NC_CACHE[key]


def _make_in_maps(x, boundary_score, W_qkv, b_qkv, W_out):
    bf16 = ml_dtypes.bfloat16
    x = np.asarray(x, np.float32)
    boundary_score = np.asarray(boundary_score, np.float32)
    W_qkv = np.asarray(W_qkv, np.float32)
    b_qkv = np.asarray(b_qkv, np.float32)
    W_out = np.asarray(W_out, np.float32)

    Wq, Wk, Wv = W_qkv[:, :D], W_qkv[:, D:2 * D], W_qkv[:, 2 * D:]
    bq, bk, bv = b_qkv[:D], b_qkv[D:2 * D], b_qkv[2 * D:]
    ones = np.ones((1, 512), bf16)
    ones65 = np.ones((65, 64), bf16)
    xts = [np.ascontiguousarray(x[b].T.astype(bf16)) for b in range(x.shape[0])]

    in_maps = []
    for c in range(8):
        b, g = divmod(c, 4)
        lo, hi = 256 * g, 256 * (g + 1)
        wqk = np.ascontiguousarray(
            np.concatenate([Wq[:, lo:hi] * SCALE, Wk[:, lo:hi]], axis=1).astype(bf16)
        )
        bqk = np.concatenate([bq[lo:hi] * SCALE, bk[lo:hi]])[None].astype(bf16)
        wv = np.ascontiguousarray(Wv[:, lo:hi].astype(bf16))
        bvv = np.ascontiguousarray(bv[lo:hi][None].astype(bf16))
        wo = np.ascontiguousarray(W_out[lo:hi, :].astype(bf16))
        bs = np.ascontiguousarray(
            (boundary_score[b] * BIAS_COEF).reshape(NKB, 128).T
        )
        in_maps.append(
            dict(
                xt=xts[b], wqk=wqk, bqk=np.ascontiguousarray(bqk),
                wv=wv, bv=bvv, wo=wo, bs=bs, ones=ones, ones65=ones65,
            )
        )
    return in_maps


def kernel(x, boundary_score, W_qkv, b_qkv, W_out, b_out):
    from concourse.bass_utils import run_bass_kernel_spmd

    x = np.asarray(x, np.float32)
    B = x.shape[0]
    in_maps = _make_in_maps(x, boundary_score, W_qkv, b_qkv, W_out)
    nc = _get_nc(with_qkv_bias=bool(np.any(np.asarray(b_qkv))))
    res = run_bass_kernel_spmd(nc, in_maps, list(range(8))).results
    out = np.zeros((B, T, D), np.float32)
    for c in range(8):
        out[c // 4] += np.asarray(res[c]["y"], np.float32)
    out += np.asarray(b_out, np.float32)
    return out
